# revision 10
# baseline (speedup 1.0000x reference)
"""Trainium2 Bass kernel for nn_COLoss_45457933860953.

Loss = mean over all pixels of weighted -log(conf gathered by instance)
     + mean over batches of (masked offset MSE sum / fg count).

Data-parallel over the batch dim: 16 batches -> 8 cores x 2 batches.

Host packs per-core inputs into two f32 tensors so each chunk-set needs
only two DMAs (walrus allows at most one semaphore wait on DVE compute
instructions, so no instruction may consume two fresh DMA lanes):
  a: [2, 3, H, W] = concat(confidence[2ch], instance bits[1ch])
  b: [2, 4, H, W] = concat(offset[2ch], gt_offset[2ch])

All per-set partial sums go to dedicated [128,1] tiles (no chained
accumulators, no buffer reuse) to keep every compute instruction at
<=1 semaphore wait; a final tree-add assembles the [128, 6] output:
  col 0: sum log(g)          (both batches)
  col 1: sum m*log(g)        (both batches)
  col 2: sum m*((g0-o0)^2 + (g1-o1)^2) batch 0
  col 3: same, batch 1
  col 4: count(m) batch 0
  col 5: count(m) batch 1
Host combines in float64:
  conf_loss = -(0.4*S1 + 0.6*S2)/N        (weight = 0.4 + 0.6*m)
  off_loss  = mean_b(sums_b / counts_b)
"""

import sys

if "/opt/trn_rl_repo" not in sys.path:
    sys.path.insert(0, "/opt/trn_rl_repo")

import numpy as np

import concourse.bass as bass
import concourse.tile as tile
from concourse import mybir
from concourse.bass_utils import run_bass_kernel_spmd

B, C, H, W = 16, 2, 512, 512
NCORES = 8
BPC = B // NCORES            # batches per core
P = 128                      # SBUF partitions
FREE = (H * W) // P          # 2048 free elems per partition per image
T = 1024                     # chunk size along the free dim
NCHUNK = FREE // T
NSETS = BPC * NCHUNK         # chunk-sets per core

F32 = mybir.dt.float32
I32 = mybir.dt.int32
AF = mybir.ActivationFunctionType
ALU = mybir.AluOpType


def _legalize_single_wait(nc):
    """This toolchain's walrus accepts at most ONE sync-wait on TPB compute
    instructions and rejects the EVENT_SEMAPHORE_RANGE_CLEAR InstISA that
    TileContext emits in its kernel tail. Drop the range clear (sems are
    not recycled in a one-shot NEFF) and hoist surplus waits onto
    standalone single-wait InstEventSemaphore carriers placed immediately
    before the instruction on the same engine queue (prefix waits on an
    in-order queue are semantically identical to instruction waits)."""
    cnt = 0
    for f in nc.m.functions:
        for blk in f.blocks:
            out = []
            for ins in blk.instructions:
                nm = type(ins).__name__
                if (nm == "InstISA" and
                        getattr(ins, "op_name", None) ==
                        "EVENT_SEMAPHORE_RANGE_CLEAR"):
                    continue
                si = getattr(ins, "sync_info", None)
                if si is not None and si.on_wait and len(si.on_wait) > 1:
                    waits = list(si.on_wait)
                    for w in waits[:-1]:
                        cnt += 1
                        out.append(mybir.InstEventSemaphore(
                            name=f"{ins.name}-hoist{cnt}",
                            engine=ins.engine,
                            ins=[], outs=[],
                            sync_info=mybir.SyncInfo(on_wait=[w],
                                                     on_update=[]),
                        ))
                    ins.sync_info = mybir.SyncInfo(
                        on_wait=[waits[-1]], on_update=list(si.on_update))
                out.append(ins)
            blk.instructions = out
    return nc


def build_nc(legalize=True):
    nc = bass.Bass("TRN2", target_bir_lowering=False, debug=False,
                   num_devices=NCORES)
    a = nc.dram_tensor("a", [BPC, 3, H, W], F32, kind="ExternalInput")
    bt = nc.dram_tensor("b", [BPC, 4, H, W], F32, kind="ExternalInput")
    out = nc.dram_tensor("partials", [P, 6], F32, kind="ExternalOutput")

    # [b, c, (p q), w] -> [p, b, c, (q w)]: partition p holds 4 contiguous
    # image rows (8KB); any column slice is contiguous per partition.
    a_r = a.rearrange("b c (p q) w -> p b c (q w)", p=P)
    b_r = bt.rearrange("b c (p q) w -> p b c (q w)", p=P)

    def acc_tiles(pool, base, n):
        return [pool.tile([P, 1], F32, name=f"{base}{i}", tag=f"{base}{i}")
                for i in range(n)]

    with tile.TileContext(nc) as tc:
        with (
            tc.tile_pool(name="io", bufs=3) as io,
            tc.tile_pool(name="work", bufs=NSETS) as work,
            tc.tile_pool(name="acc", bufs=1) as accp,
        ):
            lg_s = acc_tiles(accp, "lg_s", NSETS)     # sum log(g) per set
            mlg_s = acc_tiles(accp, "mlg_s", NSETS)   # sum m*log(g) per set
            cnt_s = acc_tiles(accp, "cnt_s", NSETS)   # count(m) per set
            off_s = acc_tiles(accp, "off_s", NSETS * C)  # per set+channel
            zb = accp.tile([P, 1], F32)               # zero bias for ACT

            nc.vector.memset(zb[:], 0.0)

            for bi in range(BPC):
                for j in range(NCHUNK):
                    si = bi * NCHUNK + j
                    cs = slice(j * T, (j + 1) * T)

                    a_t = io.tile([P, 3, T], F32)
                    nc.sync.dma_start(a_t[:], a_r[:, bi, :, cs])
                    b_t = io.tile([P, 4, T], F32)
                    nc.sync.dma_start(b_t[:], b_r[:, bi, :, cs])

                    mask = a_t[:, 2, :].bitcast(I32)

                    # m as f32; free-axis accumulation gives the fg count.
                    instf = work.tile([P, T], F32)
                    nc.scalar.activation(instf[:], mask, AF.Copy,
                                         accum_out=cnt_s[si][:])

                    # g = where(m, conf1, conf0)
                    g = work.tile([P, T], F32)
                    nc.vector.tensor_copy(g[:], a_t[:, 0, :])
                    nc.vector.copy_predicated(g[:], mask, a_t[:, 1, :])
                    # g <- log(g); free accumulation gives sum log(g)
                    nc.scalar.activation(g[:], g[:], AF.Ln, bias=zb[:],
                                         accum_out=lg_s[si][:])
                    nc.vector.scalar_tensor_tensor(
                        out=g[:], in0=g[:], scalar=1.0, in1=instf[:],
                        op0=ALU.mult, op1=ALU.mult,
                        accum_out=mlg_s[si][:])

                    for c in range(C):
                        d = work.tile([P, T], F32, name=f"d{c}", tag=f"d{c}")
                        nc.vector.tensor_sub(d[:], b_t[:, 2 + c, :],
                                             b_t[:, c, :])
                        nc.scalar.activation(d[:], d[:], AF.Square,
                                             bias=zb[:])
                        nc.vector.scalar_tensor_tensor(
                            out=d[:], in0=d[:], scalar=1.0, in1=instf[:],
                            op0=ALU.mult, op1=ALU.mult,
                            accum_out=off_s[si * C + c][:])

            res = accp.tile([P, 6], F32)

            def tree_sum(dst, tiles):
                nc.vector.tensor_add(dst, tiles[0][:], tiles[1][:])
                for t in tiles[2:]:
                    nc.vector.tensor_add(dst, dst, t[:])

            tree_sum(res[:, 0:1], lg_s)
            tree_sum(res[:, 1:2], mlg_s)
            for bi in range(BPC):
                tree_sum(res[:, 2 + bi:3 + bi],
                         off_s[bi * NCHUNK * C:(bi + 1) * NCHUNK * C])
                tree_sum(res[:, 4 + bi:5 + bi],
                         cnt_s[bi * NCHUNK:(bi + 1) * NCHUNK])
            nc.sync.dma_start(out[:, :], res[:])

    return _legalize_single_wait(nc) if legalize else nc


_NC = None


def _get_nc():
    global _NC
    if _NC is None:
        _NC = build_nc()
    return _NC


def make_in_maps(confidence, offset, instance, gt_offset):
    confidence = np.ascontiguousarray(confidence, dtype=np.float32)
    offset = np.ascontiguousarray(offset, dtype=np.float32)
    instance = np.ascontiguousarray(instance, dtype=np.int32)
    gt_offset = np.ascontiguousarray(gt_offset, dtype=np.float32)
    in_maps = []
    for k in range(NCORES):
        sl = slice(BPC * k, BPC * (k + 1))
        a = np.concatenate(
            [confidence[sl], instance[sl].view(np.float32)], axis=1)
        b = np.concatenate([offset[sl], gt_offset[sl]], axis=1)
        in_maps.append({"a": a, "b": b})
    return in_maps


def combine_partials(parts):
    """parts: list of 8 arrays [128, 6] -> scalar loss (float64)."""
    s1 = sum(p[:, 0].sum(dtype=np.float64) for p in parts)
    s2 = sum(p[:, 1].sum(dtype=np.float64) for p in parts)
    n = float(B * H * W)
    conf_loss = -(0.4 * s1 + 0.6 * s2) / n
    off_loss = 0.0
    for p in parts:
        for bi in range(BPC):
            s = p[:, 2 + bi].sum(dtype=np.float64)
            cnt = p[:, 4 + bi].sum(dtype=np.float64)
            if cnt > 0.5:
                off_loss += s / cnt
    off_loss /= B
    return conf_loss + off_loss


def kernel(confidence, offset, instance, gt_offset):
    nc = _get_nc()
    in_maps = make_in_maps(confidence, offset, instance, gt_offset)
    res = run_bass_kernel_spmd(nc, in_maps, core_ids=list(range(NCORES)))
    parts = [r["partials"] for r in res.results]
    return np.array(combine_partials(parts), dtype=np.float32)
